# revision 9
# baseline (speedup 1.0000x reference)
"""ClusterLoss (mean-entropy + batch-entropy) Bass kernel for 8 trn2 cores.

Problem: block_feats [T=4096, M*K=64*256] f32.
  x = reshape(T, M, K)
  L1 = mean over (T, M) of entropy(softmax(x, axis=K))
  L2 = -sum_m entropy(softmax(mean_t x)) / M
  out = L1 + L2   (scalar)

Sharding: columns across 8 cores (each core: 8 blocks x all 4096 rows),
and each core's slice is HOST-TRANSPOSED so K sits on partitions:
per-core DRAM x is [2048, 4096] = [(m,h,p), t] with m=block, h=K-half,
p=partition (k = h*128+p), t=row.

v4 K-on-partitions design: the per-(row,block) reductions s=sum_k exp and
u=sum_k x*exp become PARTITION reductions done on the idle PE via one-hot
matmuls, freeing DVE of the 474-op segment-sum storm that bottlenecked v2:
 - DMA  : 8 tiles [128, 2, 4096] bf16 (SWDGE f32->bf16 cast loads); tiles
          0/1 split into quarters/halves to shorten pipeline ramp-in.
 - ACT  : e = exp(x) per K-half ([128,1,4096], 3.6us) -> ~59us busy, the
          engine floor (Exp+Ln act tables preloaded before data arrives).
 - DVE  : t = x*e per half (2x bf16 TT) + block-mean cols via TS+accum
          (4x mode) -> bm_sb[128, 16]; entropy-tail vector ops.
 - PE   : s and u via ones-matmuls. lhsT = Bm[:, 63-j:95-j], a [128,32]
          one-hot (col j) slice of a single shifted ones-column matrix, so
          chunk j's [1,512] colsum lands on PSUM PARTITION j. Two row
          groups: tiles 0-3 -> ps rows 0:32 (group A), tiles 4-7 -> rows
          32:64 (B), so group A's entropy tail runs mid-stream and only
          B's short tail sits on the critical path. HW-verified exact.
 - tail : L1 = ln(s)-u/s on [64,512] distributed PSUM; L2 from bm_sb via
          tiny matmuls; AllReduce [1,2]; final scalar.

Tiles 0 and 7 are processed in t-quarters (exp/TT/matmul per quarter) to
cut pipeline ramp-in and drain.

Entropy is computed without max-subtraction: inputs are N(0,1) (|x|<~6),
exp() is safe in bf16 and matches the stable reference to ~3e-4.
"""

import sys

sys.path.insert(0, "/opt/trn_rl_repo")

import numpy as np

import concourse.bass as bass
import concourse.bacc as bacc
import concourse.tile as tile
from concourse import mybir
from concourse.bass_utils import run_bass_kernel_spmd

F32 = mybir.dt.float32
BF16 = mybir.dt.bfloat16
AF = mybir.ActivationFunctionType
OP = mybir.AluOpType

# Problem constants
T = 4096            # rows (batch)
M_TOT = 64          # blocks
K = 256             # features per block
N_CORES = 8
COLS = (M_TOT * K) // N_CORES   # 2048 columns per core
M_LOC = COLS // K               # 8 blocks per core
P = 128                         # partitions
NH = 2                          # K-halves per block (K = NH * P)
NT = M_LOC                      # 8 tiles, one per local block
NCH = T // 512                  # 8 moving chunks of 512 per K-half
HT = NT // 2                    # tiles per PSUM row-group

LMBDA = 1.0

# knobs -----------------------------------------------------------------
BUF_X = 3            # rotation depth x tiles
BUF_E = 3            # rotation depth e tiles
BUF_T = 2            # rotation depth t tiles
USE_COLLECTIVE = True  # on-device AllReduce of the two partial scalars


def _absorb_deps(eng, dst_col, dep_insts):
    """Absorb cross-engine waits on `eng`'s queue before a wait-slot-limited
    instruction (e.g. SWDGE pseudo-DMA, TS/TT/activation): one tiny
    input-free write per dependency, each carrying a single sem wait,
    advancing the engine's observed vector clock."""
    from concourse.tile_rust import add_dep_helper

    for j, di in enumerate(dep_insts):
        if hasattr(eng, "memset"):
            c = eng.memset(dst_col[:, j:j + 1], 0.0)
        else:
            c = eng.memzero(dst_col[:, j:j + 1])  # ScalarE
        add_dep_helper(c.ins, di.ins, reason="absorb wait for slot-limited op")


def _absorb(eng, dst_col, src_aps):
    """Absorb cross-engine waits: tiny copies that read the freshly produced
    tiles. Each copy carries one sem wait; once the engine has waited, its
    observed vector clock covers the tick, so the following 1-wait-slot
    instructions need no cross-engine waits. dst_col slices must be disjoint
    across calls to avoid same-engine WAW sem chains."""
    for j, src in enumerate(src_aps):
        eng.tensor_copy(dst_col[:, j:j + 1], src)


def build_nc(reps: int = 1):
    assert reps == 1
    nc = bacc.Bacc("TRN2", target_bir_lowering=False, debug=False,
                   num_devices=N_CORES)
    # per-core transposed slice: [(m h p), t]
    x_dram = nc.dram_tensor("x", [COLS, T], F32, kind="ExternalInput")
    out_dram = nc.dram_tensor("out", [1, 1], F32, kind="ExternalOutput")

    from contextlib import ExitStack

    with tile.TileContext(nc) as tc, ExitStack() as ctx:
        loads = ctx.enter_context(tc.tile_pool(name="loads", bufs=BUF_X))
        es = ctx.enter_context(tc.tile_pool(name="es", bufs=BUF_E))
        ts = ctx.enter_context(tc.tile_pool(name="ts", bufs=BUF_T))
        junks = ctx.enter_context(tc.tile_pool(name="junks", bufs=2))
        singles = ctx.enter_context(tc.tile_pool(name="singles", bufs=1))
        psum = ctx.enter_context(tc.tile_pool(name="psum", bufs=1, space="PSUM"))
        dram = ctx.enter_context(tc.tile_pool(name="dram", bufs=1, space="DRAM"))

        # persistent tiles
        Bm = singles.tile([P, 127], BF16, tag="Bm")  # shifted ones-column
        nc.vector.memset(Bm, 0.0)
        nc.vector.memset(Bm[:, 63:64], 1.0)
        ones_f32 = singles.tile([P, 1], F32, tag="ones_f32")
        nc.vector.memset(ones_f32, 1.0)
        bm_sb = singles.tile([P, NH * M_LOC], F32, tag="bm_sb")  # col h*8+m
        # wait-absorber targets (disjoint columns per use)
        ab_v = singles.tile([P, 8 * NT + 16], F32, tag="ab_v")
        ab_dma = singles.tile([P, 4 * NT], F32, tag="ab_dma")
        ab_act = singles.tile([P, 6 * NT + 4], F32, tag="ab_act")

        # PSUM: s and u accumulators; rows j = (m%4)*8 + c, group A (tiles
        # 0-3) on partitions 0:32, group B (tiles 4-7) on 32:64
        ps_s = psum.tile([64, 512], F32, tag="ps_s")
        ps_u = psum.tile([64, 512], F32, tag="ps_u")

        # L1 tail tensors (halves written mid-stream / at end)
        ln_s = singles.tile([64, 512], F32, tag="ln_s")
        rs = singles.tile([64, 512], F32, tag="rs")
        qq = singles.tile([64, 512], F32, tag="qq")
        ent_junk = singles.tile([64, 512], F32, tag="ent_junk")
        l1p = singles.tile([64, 1], F32, tag="l1p")

        x_view = x_dram.ap().rearrange("(m h p) t -> m p h t", p=P, h=NH)

        hist = {}

        def l1_tail_half(g):
            """Entropy tail for PSUM row-group g (0: rows 0:32, 1: 32:64)."""
            r = slice(32 * g, 32 * g + 32)
            ah = nc.scalar.activation(ln_s[r, :], ps_s[r, :], AF.Ln)
            nc.vector.reciprocal(rs[r, :], ps_s[r, :])
            nc.vector.tensor_tensor(qq[r, :], ps_u[r, :], rs[r, :],
                                    op=OP.mult)
            _absorb(nc.vector,
                    ab_v[r.start:r.start + 1, 8 * NT + g:8 * NT + g + 1],
                    [ln_s[r.start:r.start + 1, 0:1]])
            nc.vector.scalar_tensor_tensor(
                out=ent_junk[r, :], in0=ln_s[r, :], scalar=1.0, in1=qq[r, :],
                op0=OP.mult, op1=OP.subtract, accum_out=l1p[r, :])

        for m in range(NT):
            g = m // HT                      # PSUM row-group
            quartered = m in (0, NT - 1)

            # ---- WAR absorbs for recycled pool slots ----
            if m >= BUF_X:
                pv = hist[m - BUF_X]
                _absorb_deps(nc.gpsimd, ab_dma[:, 4 * m:4 * m + 2],
                             [pv["act_last"], pv["dve_last"]])
            if m >= BUF_E:
                pv = hist[m - BUF_E]
                _absorb_deps(nc.scalar, ab_act[:, 6 * m:6 * m + 2],
                             [pv["dve_last"], pv["s_last"]])
            if m >= BUF_T:
                pv = hist[m - BUF_T]
                _absorb_deps(nc.vector, ab_v[:, 8 * NT + 8 + m:8 * NT + 9 + m],
                             [pv["u_last"]])

            x_t = loads.tile([P, NH, T], BF16, tag="x_t")
            e_t = es.tile([P, NH, T], BF16, tag="e_t")
            t_t = ts.tile([P, NH, T], BF16, tag="t_t")

            def s_mms(h, cs, src, start_ok=True):
                last = None
                for c in cs:
                    j = (m % HT) * NCH + c
                    last = nc.tensor.matmul(
                        ps_s[32 * g:32 * g + 32, :],
                        Bm[:, 63 - j:95 - j],
                        src[:, h, c * 512:(c + 1) * 512],
                        start=(m % HT == 0 and h == 0 and c == 0
                               and start_ok),
                        stop=(m % HT == HT - 1 and h == NH - 1
                              and c == NCH - 1),
                    )
                return last

            def u_mms(h, cs, src):
                last = None
                for c in cs:
                    j = (m % HT) * NCH + c
                    last = nc.tensor.matmul(
                        ps_u[32 * g:32 * g + 32, :],
                        Bm[:, 63 - j:95 - j],
                        src[:, h, c * 512:(c + 1) * 512],
                        start=(m % HT == 0 and h == 0 and c == 0),
                        stop=(m % HT == HT - 1 and h == NH - 1
                              and c == NCH - 1),
                    )
                return last

            if quartered:
                # t-quarter pipeline: dma/exp/TT/matmuls per [128,2,1024]
                # quarter. Tile 0: shortens ramp-in; tile 7: shortens drain.
                dve_last = None
                for q in range(4):
                    sl = slice(q * 1024, (q + 1) * 1024)
                    if m == 0:
                        dh = nc.gpsimd.dma_start(
                            out=x_t[:, :, sl], in_=x_view[m][:, :, sl])
                        _absorb_deps(nc.scalar,
                                     ab_act[:, 6 * m + q:6 * m + q + 1],
                                     [dh])
                    elif q == 0:
                        dh = nc.gpsimd.dma_start(out=x_t[:], in_=x_view[m])
                        _absorb_deps(nc.scalar, ab_act[:, 6 * m:6 * m + 1],
                                     [dh])
                    ah = nc.scalar.activation(
                        e_t[:, :, sl], x_t[:, :, sl], AF.Exp)
                    # DVE: one TT over both halves of the quarter
                    srcs = [e_t[:, 0, sl.start:sl.start + 1]]
                    if q == 0:
                        srcs.append(x_t[:, 0, sl.start:sl.start + 1])
                    _absorb(nc.vector,
                            ab_v[:, 8 * m + 2 * q:8 * m + 2 * q + len(srcs)],
                            srcs)
                    tt = nc.vector.tensor_tensor(
                        t_t[:, :, sl], x_t[:, :, sl], e_t[:, :, sl],
                        op=OP.mult)
                    cs = (2 * q, 2 * q + 1)
                    for h in range(NH):
                        s_last = s_mms(h, cs, e_t)
                    for h in range(NH):
                        u_last = u_mms(h, cs, t_t)
                    dve_last = tt
                dma_h, act_h = dh, ah
                # block-mean col sums over the full tile
                junk = junks.tile([P, T], BF16, tag="junk")
                for h in range(NH):
                    dve_last = nc.vector.tensor_scalar(
                        out=junk[:, :], in0=x_t[:, h, :],
                        scalar1=1.0, scalar2=None, op0=OP.mult, op1=OP.add,
                        accum_out=bm_sb[:, h * M_LOC + m:h * M_LOC + m + 1])
                hist[m] = {"dma": dma_h, "act_last": act_h,
                           "dve_last": dve_last, "s_last": s_last,
                           "u_last": u_last}
            else:
                # ---- load + exp per K-half ----
                if m == 1:
                    # split DMA per K-half so exp h0 starts earlier
                    d0 = nc.gpsimd.dma_start(
                        out=x_t[:, 0:1, :], in_=x_view[m][:, 0:1, :])
                    d1 = nc.gpsimd.dma_start(
                        out=x_t[:, 1:2, :], in_=x_view[m][:, 1:2, :])
                    _absorb_deps(nc.scalar, ab_act[:, 6 * m:6 * m + 1], [d0])
                    a0 = nc.scalar.activation(
                        e_t[:, 0:1, :], x_t[:, 0:1, :], AF.Exp)
                    _absorb_deps(nc.scalar, ab_act[:, 6 * m + 1:6 * m + 2],
                                 [d1])
                    a1 = nc.scalar.activation(
                        e_t[:, 1:2, :], x_t[:, 1:2, :], AF.Exp)
                    dma_h = d1
                else:
                    dma_h = nc.gpsimd.dma_start(out=x_t[:], in_=x_view[m])
                    _absorb_deps(nc.scalar, ab_act[:, 6 * m:6 * m + 1],
                                 [dma_h])
                    a0 = nc.scalar.activation(
                        e_t[:, 0:1, :], x_t[:, 0:1, :], AF.Exp)
                    a1 = nc.scalar.activation(
                        e_t[:, 1:2, :], x_t[:, 1:2, :], AF.Exp)
                act_h = {0: a0, 1: a1}
                hist[m] = {"dma": dma_h, "act_last": a1}

                # ---- DVE: t = x*e per half, then block-mean col sums ----
                junk = junks.tile([P, T], BF16, tag="junk")
                _absorb(nc.vector, ab_v[:, 8 * m:8 * m + 2],
                        [x_t[:, 0, 0:1], e_t[:, 0, 0:1]])
                tt0 = nc.vector.tensor_tensor(
                    t_t[:, 0, :], x_t[:, 0, :], e_t[:, 0, :], op=OP.mult)
                _absorb(nc.vector, ab_v[:, 8 * m + 2:8 * m + 3],
                        [e_t[:, 1, 0:1]])
                tt1 = nc.vector.tensor_tensor(
                    t_t[:, 1, :], x_t[:, 1, :], e_t[:, 1, :], op=OP.mult)
                dve_last = tt1
                for h in range(NH):
                    dve_last = nc.vector.tensor_scalar(
                        out=junk[:, :], in0=x_t[:, h, :],
                        scalar1=1.0, scalar2=None, op0=OP.mult, op1=OP.add,
                        accum_out=bm_sb[:, h * M_LOC + m:h * M_LOC + m + 1])
                hist[m]["dve_last"] = dve_last

                # ---- PE: s from e, u from t ----
                for h in range(NH):
                    s_last = s_mms(h, range(NCH), e_t)
                for h in range(NH):
                    u_last = u_mms(h, range(NCH), t_t)
                hist[m]["s_last"] = s_last
                hist[m]["u_last"] = u_last

            # group A's entropy tail runs mid-stream, off the critical path
            if m == HT:
                l1_tail_half(0)

        l1_tail_half(1)
        ps_l1 = psum.tile([1, 1], F32, tag="ps_l1")
        nc.tensor.matmul(ps_l1[0:1, 0:1], ones_f32[0:64, :], l1p[:, :],
                         start=True, stop=True)

        # ---- tail: L2 from per-(K-half,block) batch-mean cols ----
        ebm = singles.tile([P, NH * M_LOC], F32, tag="ebm")
        nc.scalar.activation(ebm[:, :], bm_sb[:, :], AF.Exp, scale=1.0 / T)
        bms = singles.tile([P, NH * M_LOC], F32, tag="bms")
        nc.scalar.mul(bms[:, :], bm_sb[:, :], 1.0 / T)
        tbm = singles.tile([P, NH * M_LOC], F32, tag="tbm")
        _absorb(nc.vector, ab_v[0:1, 8 * NT + 2:8 * NT + 3], [ebm[0:1, 0:1]])
        nc.vector.tensor_tensor(tbm[:, :], bms[:, :], ebm[:, :], op=OP.mult)
        ps_bm = psum.tile([1, 4 * M_LOC], F32, tag="ps_bm")
        nc.tensor.matmul(ps_bm[0:1, 0:2 * M_LOC], ones_f32[:, :], ebm[:, :],
                         start=True, stop=True)
        nc.tensor.matmul(ps_bm[0:1, 2 * M_LOC:4 * M_LOC], ones_f32[:, :],
                         tbm[:, :], start=True, stop=True)
        # fold the two K-half partials per block (copy PSUM->SBUF first:
        # walrus rejects TensorTensor with two PSUM operands)
        bm4 = singles.tile([1, 4 * M_LOC], F32, tag="bm4")
        nc.scalar.copy(bm4[0:1, :], ps_bm[0:1, :])
        sm = singles.tile([1, M_LOC], F32, tag="sm")
        um = singles.tile([1, M_LOC], F32, tag="um")
        nc.vector.tensor_add(sm[0:1, :], bm4[0:1, 0:M_LOC],
                             bm4[0:1, M_LOC:2 * M_LOC])
        nc.vector.tensor_add(um[0:1, :], bm4[0:1, 2 * M_LOC:3 * M_LOC],
                             bm4[0:1, 3 * M_LOC:4 * M_LOC])
        ln_sbm = singles.tile([1, M_LOC], F32, tag="ln_sbm")
        nc.scalar.activation(ln_sbm[0:1, :], sm[0:1, :], AF.Ln)
        r_sbm = singles.tile([1, M_LOC], F32, tag="r_sbm")
        nc.vector.reciprocal(r_sbm[0:1, :], sm[0:1, :])
        q_bm = singles.tile([1, M_LOC], F32, tag="q_bm")
        nc.vector.tensor_tensor(q_bm[0:1, :], um[0:1, :], r_sbm[0:1, :],
                                op=OP.mult)
        entbm_junk = singles.tile([1, M_LOC], F32, tag="entbm_junk")
        l2p = singles.tile([1, 1], F32, tag="l2p")
        _absorb(nc.vector, ab_v[0:1, 8 * NT + 3:8 * NT + 4],
                [ln_sbm[0:1, 0:1]])
        nc.vector.scalar_tensor_tensor(
            out=entbm_junk[0:1, :], in0=ln_sbm[0:1, :], scalar=1.0,
            in1=q_bm[0:1, :], op0=OP.mult, op1=OP.subtract,
            accum_out=l2p[0:1, :])

        # ---- pack partials, AllReduce, final scalar ----
        cc_sb = singles.tile([1, 2], F32, tag="cc_sb")
        nc.scalar.copy(cc_sb[0:1, 0:1], ps_l1[0:1, 0:1])
        nc.scalar.copy(cc_sb[0:1, 1:2], l2p[0:1, 0:1])
        cc_res = singles.tile([1, 2], F32, tag="cc_res")
        if USE_COLLECTIVE:
            cc_in = dram.tile([1, 2], F32, tag="cc_in")
            cc_out = dram.tile([1, 2], F32, tag="cc_out")
            nc.gpsimd.dma_start(cc_in[:], cc_sb[:])
            nc.gpsimd.collective_compute(
                "AllReduce", OP.add,
                replica_groups=[list(range(N_CORES))],
                ins=[cc_in.opt()], outs=[cc_out.opt()])
            nc.sync.dma_start(cc_res[:], cc_out[:])
        else:
            # per-core partials only; host sums the per-core outputs
            nc.vector.tensor_copy(cc_res[:], cc_sb[:])

        t0 = singles.tile([1, 1], F32, tag="t0")
        nc.scalar.mul(t0[0:1, :], cc_res[0:1, 0:1], 1.0 / (T * M_TOT))
        t1 = singles.tile([1, 1], F32, tag="t1")
        nc.scalar.mul(t1[0:1, :], cc_res[0:1, 1:2], -LMBDA / M_TOT)
        out_sb = singles.tile([1, 1], F32, tag="out_sb")
        nc.vector.tensor_add(out_sb[0:1, :], t0[0:1, :], t1[0:1, :])
        nc.sync.dma_start(out_dram.ap(), out_sb[:])

    nc.compile()
    return nc


_NC_CACHE = None


def _get_nc():
    global _NC_CACHE
    if _NC_CACHE is None:
        _NC_CACHE = build_nc()
    return _NC_CACHE


def _run(block_feats: np.ndarray, trace: bool = False):
    nc = _get_nc()
    x = np.asarray(block_feats, dtype=np.float32)
    assert x.shape == (T, N_CORES * COLS), x.shape
    in_maps = [
        {"x": np.ascontiguousarray(x[:, c * COLS:(c + 1) * COLS].T)}
        for c in range(N_CORES)
    ]
    res = run_bass_kernel_spmd(nc, in_maps, list(range(N_CORES)), trace=trace)
    val = np.float32(res.results[0]["out"][0, 0])
    return val, res


def kernel(block_feats: np.ndarray) -> np.ndarray:
    val, _ = _run(block_feats)
    return np.array(val, dtype=np.float32)


if __name__ == "__main__":
    rng = np.random.default_rng(0)
    xf = rng.standard_normal((T, N_CORES * COLS), dtype=np.float32)
    v = kernel(xf)
    print("kernel out:", v)
